# revision 4
# baseline (speedup 1.0000x reference)
"""CorefHead Trainium2 kernel.

Reference computation (B=64, S=512, H=1024, HID=512):
  emb_a = span_mean(bert, offsets[:,0:2])   # [B,H]
  emb_b = span_mean(bert, offsets[:,2:4])   # [B,H]
  emb_p = bert[b, offsets[:,4]]             # [B,H]
  x = concat([emb_a, emb_b, emb_p], -1)     # [B,3H]
  h = leaky_relu(batchnorm_eval(x @ W1 + b1), 0.01)
  out = h @ W2 + b2                         # [B,3]

Strategy: pure data parallel, batch sharded 8 ways (8 batches/core).
Per core:
  - Host precomputes prescaled span masks (1/len) + pron one-hot, packed
    [128, nch, 3]; host also packs only the needed row-window of bert per
    batch (union of span/pron rows, padded to 128-row chunks).
  - mm1 (PE): xT[h_chunk] += bert_chunk.T @ mask_chunk -> x transposed
    [3072, 8] directly (no on-device transpose needed).
  - mm2 (PE): h[8, 512] += xT_chunk.T @ W1_chunk over 24 K-chunks.
  - BN+LeakyReLU (DVE): y = max(t, 0.01*t), t = h*scale + bias with
    scale/bias folded from (b1, gamma, beta, running stats) on host.
  - mm3 (DVE): out[:, j] = b2[j] + sum(y * W2[:, j]) via tensor_tensor_reduce.
Host gathers per-core [8, 3] outputs and undoes the batch permutation.
"""

import numpy as np

B, S, H = 64, 512, 1024
HID = 512
EPS = 1e-5
NCORES = 8
BPC = B // NCORES  # batches per core
KC = 3 * H // 128  # 24 contraction chunks for mm2
HC = H // 128      # 8 h-chunks per embedding

# Set to True to ship only the needed row-window of bert per batch.
WINDOW = True

# Test-harness hooks (harness calls kernel() with TRACE=False default).
TRACE = False
LAST_RESULT = None

_PROGRAM_CACHE: dict = {}


def _build_program(nch_slots: tuple):
    """Build + compile the SPMD Bass program for the given per-slot chunk
    counts (number of 128-row S-chunks shipped per batch slot)."""
    import concourse.bacc as bacc
    import concourse.tile as tile
    import concourse.mybir as mybir
    from concourse.bass import MemorySpace

    f32 = mybir.dt.float32
    ncht = int(sum(nch_slots))
    totrows = ncht * 128

    nc = bacc.Bacc("TRN2", target_bir_lowering=False, debug=False,
                   num_devices=NCORES)

    bert_d = nc.dram_tensor("bertw", [totrows, H], f32, kind="ExternalInput").ap()
    mask_d = nc.dram_tensor("maskp", [128, ncht, 3], f32, kind="ExternalInput").ap()
    w1_d = nc.dram_tensor("w1", [3 * H, HID], f32, kind="ExternalInput").ap()
    bns_d = nc.dram_tensor("bns", [BPC, HID], f32, kind="ExternalInput").ap()
    bnb_d = nc.dram_tensor("bnb", [BPC, HID], f32, kind="ExternalInput").ap()
    w2b_d = nc.dram_tensor("w2b", [BPC, 3, HID], f32, kind="ExternalInput").ap()
    b2b_d = nc.dram_tensor("b2b", [BPC, 3], f32, kind="ExternalInput").ap()
    out_d = nc.dram_tensor("out", [BPC, 3], f32, kind="ExternalOutput").ap()

    chbase = np.concatenate([[0], np.cumsum(nch_slots)]).astype(int)

    with tile.TileContext(nc) as tc:
        with (
            tc.tile_pool(name="singles", bufs=1) as singles,
            tc.tile_pool(name="bert_pool", bufs=3) as bert_pool,
            tc.tile_pool(name="head", bufs=1) as head,
            tc.tile_pool(name="psum_x", bufs=4, space=MemorySpace.PSUM) as psum_x_pool,
            tc.tile_pool(name="psum_h", bufs=1, space=MemorySpace.PSUM) as psum_h_pool,
        ):
            # --- constant / parameter loads ---
            mask_t = singles.tile([128, ncht, 3], f32)
            nc.sync.dma_start(out=mask_t, in_=mask_d)
            w1_t = singles.tile([128, KC, HID], f32)
            nc.sync.dma_start(
                out=w1_t, in_=w1_d.rearrange("(kc p) n -> p kc n", p=128))
            bns_t = head.tile([BPC, HID], f32)
            nc.sync.dma_start(out=bns_t, in_=bns_d)
            bnb_t = head.tile([BPC, HID], f32)
            nc.sync.dma_start(out=bnb_t, in_=bnb_d)
            w2b_t = head.tile([BPC, 3, HID], f32)
            nc.sync.dma_start(out=w2b_t, in_=w2b_d)
            b2b_t = head.tile([BPC, 3], f32)
            nc.sync.dma_start(out=b2b_t, in_=b2b_d)

            # xT accumulator: [128, 3 embeddings, HC chunks * BPC batches]
            # column for contraction-chunk kc=(e*HC+hc), batch b is
            # xT[:, e, hc*BPC + b]  -> mm2 rhs slice [128, BPC] contiguous.
            xT_t = singles.tile([128, 3, HC * BPC], f32)

            # --- mm1: per-batch span sums (bert chunks as PE weights) ---
            for b in range(BPC):
                nch = int(nch_slots[b])
                bt = bert_pool.tile([128, nch, H], f32, tag="bert")
                r0 = int(chbase[b]) * 128
                nc.sync.dma_start(
                    out=bt,
                    in_=bert_d[r0:r0 + nch * 128, :].rearrange(
                        "(sc p) h -> p sc h", p=128))
                for hc in range(HC):
                    px = psum_x_pool.tile([128, 3], f32)
                    for sc in range(nch):
                        nc.tensor.matmul(
                            px,
                            bt[:, sc, hc * 128:(hc + 1) * 128],
                            mask_t[:, int(chbase[b]) + sc, :],
                            start=(sc == 0),
                            stop=(sc == nch - 1),
                        )
                    nc.vector.tensor_copy(xT_t[:, :, hc * BPC + b], px)

            # --- mm2: h[BPC, HID] = x @ W1 (accumulate over 24 K-chunks) ---
            ph = psum_h_pool.tile([BPC, HID], f32)
            for kc in range(KC):
                e, hc = kc // HC, kc % HC
                nc.tensor.matmul(
                    ph,
                    xT_t[:, e, hc * BPC:(hc + 1) * BPC],
                    w1_t[:, kc, :],
                    start=(kc == 0),
                    stop=(kc == KC - 1),
                )

            # --- BN (eval, folded) + LeakyReLU(0.01) ---
            t1 = head.tile([BPC, HID], f32)
            nc.vector.tensor_mul(t1, ph, bns_t)
            t2 = head.tile([BPC, HID], f32)
            nc.vector.tensor_add(t2, t1, bnb_t)
            y_t = head.tile([BPC, HID], f32)
            # y = max(0.01 * t2, t2)
            nc.vector.scalar_tensor_tensor(
                y_t, t2, 0.01, t2,
                op0=mybir.AluOpType.mult, op1=mybir.AluOpType.max)

            # --- mm3: out[:, j] = b2[j] + sum(y * W2[:, j]) ---
            # (tensor_tensor_reduce crashes the exec unit on this runtime;
            # use mul + reduce_sum instead)
            osum = head.tile([BPC, 3], f32)
            scratch = head.tile([BPC, HID], f32)
            for j in range(3):
                nc.vector.tensor_mul(scratch, y_t, w2b_t[:, j, :])
                nc.vector.reduce_sum(out=osum[:, j:j + 1], in_=scratch,
                                     axis=mybir.AxisListType.X)
            o_t = head.tile([BPC, 3], f32)
            nc.vector.tensor_add(o_t, osum, b2b_t)

            nc.sync.dma_start(out=out_d, in_=o_t)

    nc.compile()
    return nc


def _prep_core_inputs(bert, offsets, w1, bns, bnb, w2b, b2b, batch_idx,
                      nch_slots):
    """Build the per-core input map for the given batch indices."""
    ncht = int(sum(nch_slots))
    bertw = np.empty((ncht * 128, H), dtype=np.float32)
    maskp = np.zeros((128, ncht, 3), dtype=np.float32)
    row = 0
    for slot, gb in enumerate(batch_idx):
        nch = int(nch_slots[slot])
        L = nch * 128
        a0, a1, b0, b1_, p = (int(v) for v in offsets[gb])
        lo = min(a0, b0, p)
        w0 = max(0, min(lo, S - L))
        bertw[row:row + L] = bert[gb, w0:w0 + L]
        pos = w0 + np.arange(L)
        ma = ((pos >= a0) & (pos <= a1)).astype(np.float32) / (a1 - a0 + 1)
        mb = ((pos >= b0) & (pos <= b1_)).astype(np.float32) / (b1_ - b0 + 1)
        mp = (pos == p).astype(np.float32)
        blk = np.stack([ma, mb, mp], axis=-1).reshape(nch, 128, 3)
        maskp[:, row // 128:row // 128 + nch, :] = blk.transpose(1, 0, 2)
        row += L
    return {
        "bertw": bertw,
        "maskp": maskp,
        "w1": w1,
        "bns": bns,
        "bnb": bnb,
        "w2b": w2b,
        "b2b": b2b,
    }


def kernel(bert_outputs, offsets, W1, b1, gamma, beta, running_mean,
           running_var, W2, b2):
    bert = np.ascontiguousarray(np.asarray(bert_outputs, dtype=np.float32))
    offs = np.asarray(offsets).astype(np.int64)
    W1 = np.ascontiguousarray(np.asarray(W1, dtype=np.float32))
    b1 = np.asarray(b1, dtype=np.float32)
    gamma = np.asarray(gamma, dtype=np.float32)
    beta = np.asarray(beta, dtype=np.float32)
    rm = np.asarray(running_mean, dtype=np.float32)
    rv = np.asarray(running_var, dtype=np.float32)
    W2 = np.asarray(W2, dtype=np.float32)
    b2 = np.asarray(b2, dtype=np.float32)

    # Fold BN eval-mode stats: bn(xW1 + b1) = (xW1)*s + ((b1 - mean)*s + beta)
    s = gamma / np.sqrt(rv + EPS)
    bias = (b1 - rm) * s + beta
    bns = np.broadcast_to(s, (BPC, HID)).copy()
    bnb = np.broadcast_to(bias, (BPC, HID)).copy()
    w2b = np.broadcast_to(W2.T[None, :, :], (BPC, 3, HID)).copy()
    b2b = np.broadcast_to(b2, (BPC, 3)).copy()

    # Row windows: union of span/pron rows per batch, padded to 128-row
    # chunks. Sort batches by window size so same-slot batches across cores
    # share one (max) chunk count; undone at gather time.
    if WINDOW:
        lo = offs[:, [0, 2, 4]].min(axis=1)
        hi = offs[:, [1, 3, 4]].max(axis=1)
        lens = np.minimum((hi - lo + 128) // 128 * 128, S)
        order = np.argsort(-lens, kind="stable")
    else:
        lens = np.full(B, S, dtype=np.int64)
        order = np.arange(B)

    # slot i of every core holds batches ranked [i*NCORES, (i+1)*NCORES)
    perm = order.reshape(BPC, NCORES)  # [slot, core] -> global batch
    nch_slots = tuple(int(lens[perm[i]].max()) // 128 for i in range(BPC))

    key = nch_slots
    if key not in _PROGRAM_CACHE:
        _PROGRAM_CACHE[key] = _build_program(key)
    nc = _PROGRAM_CACHE[key]

    in_maps = [
        _prep_core_inputs(bert, offs, W1, bns, bnb, w2b, b2b,
                          perm[:, c], nch_slots)
        for c in range(NCORES)
    ]

    from concourse import bass_utils
    kwargs = {}
    if TRACE:
        kwargs = {"trace": True, "trace_cores": list(range(NCORES))}
    res = bass_utils.run_bass_kernel_spmd(nc, in_maps,
                                          core_ids=list(range(NCORES)),
                                          **kwargs)
    global LAST_RESULT
    LAST_RESULT = res

    out = np.empty((B, 3), dtype=np.float32)
    for c in range(NCORES):
        out[perm[:, c]] = res.results[c]["out"]
    return out


# revision 19
# speedup vs baseline: 1.8074x; 1.8074x over previous
"""CorefHead Trainium2 kernel.

Reference computation (B=64, S=512, H=1024, HID=512):
  emb_a = span_mean(bert, offsets[:,0:2])   # [B,H]
  emb_b = span_mean(bert, offsets[:,2:4])   # [B,H]
  emb_p = bert[b, offsets[:,4]]             # [B,H]
  x = concat([emb_a, emb_b, emb_p], -1)     # [B,3H]
  h = leaky_relu(batchnorm_eval(x @ W1 + b1), 0.01)
  out = h @ W2 + b2                         # [B,3]

Strategy: pure data parallel, batch sharded 8 ways (8 batches/core).
Per core:
  - Host precomputes prescaled span masks (1/len) + pron one-hot, packed
    [128, nch, 3]; host also packs only the needed row-window of bert per
    batch (union of span/pron rows, padded to 128-row chunks).
  - mm1 (PE): xT[h_chunk] += bert_chunk.T @ mask_chunk -> x transposed
    [3072, 8] directly (no on-device transpose needed).
  - mm2 (PE): h[8, 512] += xT_chunk.T @ W1_chunk over 24 K-chunks.
  - BN+LeakyReLU (DVE): y = max(t, 0.01*t), t = h*scale + bias with
    scale/bias folded from (b1, gamma, beta, running stats) on host.
  - mm3 (DVE): out[:, j] = b2[j] + sum(y * W2[:, j]) via tensor_tensor_reduce.
Host gathers per-core [8, 3] outputs and undoes the batch permutation.
"""

import numpy as np

B, S, H = 64, 512, 1024
HID = 512
EPS = 1e-5
NCORES = 8
BPC = B // NCORES  # batches per core
KC = 3 * H // 128  # 24 contraction chunks for mm2
HC = H // 128      # 8 h-chunks per embedding

# Set to True to ship only the needed row-window of bert per batch.
WINDOW = True
# bert/masks in bf16: halves the dominant DMA traffic and runs mm1 single
# pass with fast weight load on the PE. Masks hold exact 0/1 (bf16-exact);
# the 1/span_len scaling happens in fp32 on the PSUM->SBUF copy.
BERT_BF16 = True
# W1 (and the mm2 x operand) in bf16.
W1_BF16 = False

# Test-harness hooks (harness calls kernel() with TRACE=False default).
TRACE = False
LAST_RESULT = None

_PROGRAM_CACHE: dict = {}


def _build_program(nch_slots: tuple):
    """Build + compile the SPMD Bass program for the given per-slot chunk
    counts (number of 128-row S-chunks shipped per batch slot)."""
    import concourse.bacc as bacc
    import concourse.tile as tile
    import concourse.mybir as mybir
    from concourse.bass import MemorySpace

    f32 = mybir.dt.float32
    bdt = mybir.dt.bfloat16 if BERT_BF16 else f32
    wdt = mybir.dt.bfloat16 if W1_BF16 else f32
    ncht = int(sum(nch_slots))
    totrows = ncht * 128

    nc = bacc.Bacc("TRN2", target_bir_lowering=False, debug=False,
                   num_devices=NCORES)

    bert_d = nc.dram_tensor("bertw", [totrows, H], bdt, kind="ExternalInput").ap()
    mask_d = nc.dram_tensor("maskp", [128, ncht, 2], bdt, kind="ExternalInput").ap()
    sfac_d = nc.dram_tensor("sfac", [128, BPC, 2], f32, kind="ExternalInput").ap()
    pron_d = nc.dram_tensor("pron", [BPC, H], f32, kind="ExternalInput").ap()
    w1_d = nc.dram_tensor("w1", [3 * H, HID], wdt, kind="ExternalInput").ap()
    bns_d = nc.dram_tensor("bns", [BPC, HID], f32, kind="ExternalInput").ap()
    bnb_d = nc.dram_tensor("bnb", [BPC, HID], f32, kind="ExternalInput").ap()
    w2b_d = nc.dram_tensor("w2b", [BPC, 3, HID], f32, kind="ExternalInput").ap()
    b2b_d = nc.dram_tensor("b2b", [BPC, 3], f32, kind="ExternalInput").ap()
    out_d = nc.dram_tensor("out", [BPC, 3], f32, kind="ExternalOutput").ap()

    chbase = np.concatenate([[0], np.cumsum(nch_slots)]).astype(int)

    with tile.TileContext(nc) as tc:
        with (
            tc.tile_pool(name="singles", bufs=1) as singles,
            tc.tile_pool(name="bert_pool", bufs=3) as bert_pool,
            tc.tile_pool(name="head", bufs=1) as head,
            tc.tile_pool(name="psum_x", bufs=4, space=MemorySpace.PSUM) as psum_x_pool,
            tc.tile_pool(name="psum_p", bufs=2, space=MemorySpace.PSUM) as psum_p_pool,
            tc.tile_pool(name="psum_h", bufs=1, space=MemorySpace.PSUM) as psum_h_pool,
        ):
            # --- constant / parameter loads ---
            mask_t = singles.tile([128, ncht, 2], bdt)
            nc.sync.dma_start(out=mask_t, in_=mask_d)
            sfac_t = singles.tile([128, BPC, 2], f32)
            nc.sync.dma_start(out=sfac_t, in_=sfac_d)
            pron_t = singles.tile([BPC, H], f32)
            nc.sync.dma_start(out=pron_t, in_=pron_d)
            w1_t = singles.tile([128, KC, HID], wdt)
            nc.sync.dma_start(
                out=w1_t, in_=w1_d.rearrange("(kc p) n -> p kc n", p=128))
            bns_t = head.tile([BPC, HID], f32)
            nc.sync.dma_start(out=bns_t, in_=bns_d)
            bnb_t = head.tile([BPC, HID], f32)
            nc.sync.dma_start(out=bnb_t, in_=bnb_d)
            w2b_t = head.tile([BPC, 3, HID], f32)
            nc.sync.dma_start(out=w2b_t, in_=w2b_d)
            b2b_t = head.tile([BPC, 3], f32)
            nc.sync.dma_start(out=b2b_t, in_=b2b_d)

            # xT accumulator: [128, 3 embeddings, HC chunks * BPC batches]
            # column for contraction-chunk kc=(e*HC+hc), batch b is
            # xT[:, e, hc*BPC + b]  -> mm2 rhs slice [128, BPC] contiguous.
            xT_t = singles.tile([128, 3, HC * BPC], wdt)

            # --- pron embedding: exact fp32 rows, transposed via PE ---
            from concourse.masks import make_identity
            idt = singles.tile([BPC, BPC], f32)
            make_identity(nc, idt)
            for hc in range(HC):
                pxp = psum_p_pool.tile([128, BPC], f32, tag="pxp")
                nc.tensor.transpose(pxp, pron_t[:, hc * 128:(hc + 1) * 128], idt)
                nc.vector.tensor_copy(xT_t[:, 2, hc * BPC:(hc + 1) * BPC], pxp)

            # --- mm1: per-batch span sums (bert chunks as PE weights) ---
            for b in range(BPC):
                nch = int(nch_slots[b])
                bt = bert_pool.tile([128, nch, H], bdt, tag="bert")
                r0 = int(chbase[b]) * 128
                nc.sync.dma_start(
                    out=bt,
                    in_=bert_d[r0:r0 + nch * 128, :].rearrange(
                        "(sc p) h -> p sc h", p=128))
                for hc in range(HC):
                    px = psum_x_pool.tile([128, 2], f32)
                    for sc in range(nch):
                        nc.tensor.matmul(
                            px,
                            bt[:, sc, hc * 128:(hc + 1) * 128],
                            mask_t[:, int(chbase[b]) + sc, :],
                            start=(sc == 0),
                            stop=(sc == nch - 1),
                        )
                    # fp32 scale by (1/lenA, 1/lenB) during PSUM->SBUF
                    nc.vector.tensor_mul(
                        xT_t[:, 0:2, hc * BPC + b], px, sfac_t[:, b, :])

            # --- mm2: h[BPC, HID] = x @ W1 (accumulate over 24 K-chunks) ---
            ph = psum_h_pool.tile([BPC, HID], f32)
            for kc in range(KC):
                e, hc = kc // HC, kc % HC
                nc.tensor.matmul(
                    ph,
                    xT_t[:, e, hc * BPC:(hc + 1) * BPC],
                    w1_t[:, kc, :],
                    start=(kc == 0),
                    stop=(kc == KC - 1),
                )

            # --- BN (eval, folded) + LeakyReLU(0.01) ---
            t1 = head.tile([BPC, HID], f32)
            nc.vector.tensor_mul(t1, ph, bns_t)
            t2 = head.tile([BPC, HID], f32)
            nc.vector.tensor_add(t2, t1, bnb_t)
            y_t = head.tile([BPC, HID], f32)
            # y = max(0.01 * t2, t2)
            nc.vector.scalar_tensor_tensor(
                y_t, t2, 0.01, t2,
                op0=mybir.AluOpType.mult, op1=mybir.AluOpType.max)

            # --- mm3: out[:, j] = b2[j] + sum(y * W2[:, j]) ---
            # (tensor_tensor_reduce crashes the exec unit on this runtime;
            # use mul + reduce_sum instead)
            osum = head.tile([BPC, 3], f32)
            scratch = head.tile([BPC, HID], f32)
            for j in range(3):
                nc.vector.tensor_mul(scratch, y_t, w2b_t[:, j, :])
                nc.vector.reduce_sum(out=osum[:, j:j + 1], in_=scratch,
                                     axis=mybir.AxisListType.X)
            o_t = head.tile([BPC, 3], f32)
            nc.vector.tensor_add(o_t, osum, b2b_t)

            nc.sync.dma_start(out=out_d, in_=o_t)

    nc.compile()
    return nc


def _prep_core_inputs(bert, bert_f32, offsets, w1, bns, bnb, w2b, b2b,
                      batch_idx, nch_slots):
    """Build the per-core input map for the given batch indices."""
    ncht = int(sum(nch_slots))
    bertw = np.empty((ncht * 128, H), dtype=bert.dtype)
    maskp = np.zeros((128, ncht, 2), dtype=bert.dtype)
    sfac = np.ones((BPC, 2), dtype=np.float32)
    pron = np.empty((BPC, H), dtype=np.float32)
    row = 0
    for slot, gb in enumerate(batch_idx):
        nch = int(nch_slots[slot])
        L = nch * 128
        a0, a1, b0, b1_, p = (int(v) for v in offsets[gb])
        lo = min(a0, b0, p)
        w0 = max(0, min(lo, S - L))
        bertw[row:row + L] = bert[gb, w0:w0 + L]
        pron[slot] = bert_f32[gb, p]
        pos = w0 + np.arange(L)
        ma = ((pos >= a0) & (pos <= a1)).astype(np.float32)
        mb = ((pos >= b0) & (pos <= b1_)).astype(np.float32)
        sfac[slot, 0] = 1.0 / (a1 - a0 + 1)
        sfac[slot, 1] = 1.0 / (b1_ - b0 + 1)
        blk = np.stack([ma, mb], axis=-1).reshape(nch, 128, 2)
        maskp[:, row // 128:row // 128 + nch, :] = blk.transpose(1, 0, 2)
        row += L
    return {
        "bertw": bertw,
        "maskp": maskp,
        "sfac": np.broadcast_to(sfac, (128, BPC, 2)).copy(),
        "pron": pron,
        "w1": w1,
        "bns": bns,
        "bnb": bnb,
        "w2b": w2b,
        "b2b": b2b,
    }


def kernel(bert_outputs, offsets, W1, b1, gamma, beta, running_mean,
           running_var, W2, b2):
    import ml_dtypes

    bert_f32 = np.ascontiguousarray(np.asarray(bert_outputs, dtype=np.float32))
    bert = bert_f32.astype(ml_dtypes.bfloat16) if BERT_BF16 else bert_f32
    offs = np.asarray(offsets).astype(np.int64)
    W1 = np.ascontiguousarray(np.asarray(W1, dtype=np.float32))
    if W1_BF16:
        W1 = W1.astype(ml_dtypes.bfloat16)
    b1 = np.asarray(b1, dtype=np.float32)
    gamma = np.asarray(gamma, dtype=np.float32)
    beta = np.asarray(beta, dtype=np.float32)
    rm = np.asarray(running_mean, dtype=np.float32)
    rv = np.asarray(running_var, dtype=np.float32)
    W2 = np.asarray(W2, dtype=np.float32)
    b2 = np.asarray(b2, dtype=np.float32)

    # Fold BN eval-mode stats: bn(xW1 + b1) = (xW1)*s + ((b1 - mean)*s + beta)
    s = gamma / np.sqrt(rv + EPS)
    bias = (b1 - rm) * s + beta
    bns = np.broadcast_to(s, (BPC, HID)).copy()
    bnb = np.broadcast_to(bias, (BPC, HID)).copy()
    w2b = np.broadcast_to(W2.T[None, :, :], (BPC, 3, HID)).copy()
    b2b = np.broadcast_to(b2, (BPC, 3)).copy()

    # Row windows: union of span/pron rows per batch, padded to 128-row
    # chunks. Sort batches by window size so same-slot batches across cores
    # share one (max) chunk count; undone at gather time.
    if WINDOW:
        lo = offs[:, [0, 2, 4]].min(axis=1)
        hi = offs[:, [1, 3, 4]].max(axis=1)
        lens = np.minimum((hi - lo + 128) // 128 * 128, S)
        order = np.argsort(-lens, kind="stable")
    else:
        lens = np.full(B, S, dtype=np.int64)
        order = np.arange(B)

    # slot i of every core holds batches ranked [i*NCORES, (i+1)*NCORES)
    perm = order.reshape(BPC, NCORES)  # [slot, core] -> global batch
    nch_slots = tuple(int(lens[perm[i]].max()) // 128 for i in range(BPC))

    key = nch_slots
    if key not in _PROGRAM_CACHE:
        _PROGRAM_CACHE[key] = _build_program(key)
    nc = _PROGRAM_CACHE[key]

    in_maps = [
        _prep_core_inputs(bert, bert_f32, offs, W1, bns, bnb, w2b, b2b,
                          perm[:, c], nch_slots)
        for c in range(NCORES)
    ]

    from concourse import bass_utils
    kwargs = {}
    if TRACE:
        kwargs = {"trace": True, "trace_cores": list(range(NCORES))}
    res = bass_utils.run_bass_kernel_spmd(nc, in_maps,
                                          core_ids=list(range(NCORES)),
                                          **kwargs)
    global LAST_RESULT
    LAST_RESULT = res

    out = np.empty((B, 3), dtype=np.float32)
    for c in range(NCORES):
        out[perm[:, c]] = res.results[c]["out"]
    return out


# revision 23
# speedup vs baseline: 2.4472x; 1.3540x over previous
"""CorefHead Trainium2 kernel.

Reference computation (B=64, S=512, H=1024, HID=512):
  emb_a = span_mean(bert, offsets[:,0:2])   # [B,H]
  emb_b = span_mean(bert, offsets[:,2:4])   # [B,H]
  emb_p = bert[b, offsets[:,4]]             # [B,H]
  x = concat([emb_a, emb_b, emb_p], -1)     # [B,3H]
  h = leaky_relu(batchnorm_eval(x @ W1 + b1), 0.01)
  out = h @ W2 + b2                         # [B,3]

Strategy: pure data parallel, batch sharded 8 ways (8 batches/core).
Per core:
  - Host precomputes prescaled span masks (1/len) + pron one-hot, packed
    [128, nch, 3]; host also packs only the needed row-window of bert per
    batch (union of span/pron rows, padded to 128-row chunks).
  - mm1 (PE): xT[h_chunk] += bert_chunk.T @ mask_chunk -> x transposed
    [3072, 8] directly (no on-device transpose needed).
  - mm2 (PE): h[8, 512] += xT_chunk.T @ W1_chunk over 24 K-chunks.
  - BN+LeakyReLU (DVE): y = max(t, 0.01*t), t = h*scale + bias with
    scale/bias folded from (b1, gamma, beta, running stats) on host.
  - mm3 (DVE): out[:, j] = b2[j] + sum(y * W2[:, j]) via tensor_tensor_reduce.
Host gathers per-core [8, 3] outputs and undoes the batch permutation.
"""

import numpy as np

B, S, H = 64, 512, 1024
HID = 512
EPS = 1e-5
NCORES = 8
BPC = B // NCORES  # batches per core
KC = 3 * H // 128  # 24 contraction chunks for mm2
HC = H // 128      # 8 h-chunks per embedding

# Set to True to ship only the needed row-window of bert per batch.
WINDOW = True
# bert/masks in bf16: halves the dominant DMA traffic and runs mm1 single
# pass with fast weight load on the PE. Masks hold exact 0/1 (bf16-exact);
# the 1/span_len scaling happens in fp32 on the PSUM->SBUF copy.
BERT_BF16 = True
# W1 (and the mm2 x operand) in bf16.
W1_BF16 = True
# Ship pron rows fp32 + transpose on device. Only buys precision when the
# mm2 operands stay fp32 (otherwise x is rounded to bf16 anyway).
PRON_FP32 = not W1_BF16

# Test-harness hooks (harness calls kernel() with TRACE=False default).
TRACE = False
LAST_RESULT = None

_PROGRAM_CACHE: dict = {}


def _build_program(nch_slots: tuple):
    """Build + compile the SPMD Bass program for the given per-slot chunk
    counts (number of 128-row S-chunks shipped per batch slot)."""
    import concourse.bacc as bacc
    import concourse.tile as tile
    import concourse.mybir as mybir
    from concourse.bass import MemorySpace

    f32 = mybir.dt.float32
    bdt = mybir.dt.bfloat16 if BERT_BF16 else f32
    wdt = mybir.dt.bfloat16 if W1_BF16 else f32
    ncht = int(sum(nch_slots))
    totrows = ncht * 128

    nc = bacc.Bacc("TRN2", target_bir_lowering=False, debug=False,
                   num_devices=NCORES)

    nmc = 2 if PRON_FP32 else 3  # mask columns (spans only, or spans+pron)

    bert_d = nc.dram_tensor("bertw", [totrows, H], bdt, kind="ExternalInput").ap()
    mask_d = nc.dram_tensor("maskp", [128, ncht, nmc], bdt, kind="ExternalInput").ap()
    sfac_d = nc.dram_tensor("sfac", [128, BPC, nmc], f32, kind="ExternalInput").ap()
    if PRON_FP32:
        pron_d = nc.dram_tensor("pron", [BPC, H], f32, kind="ExternalInput").ap()
    w1_d = nc.dram_tensor("w1", [3 * H, HID], wdt, kind="ExternalInput").ap()
    bns_d = nc.dram_tensor("bns", [BPC, HID], f32, kind="ExternalInput").ap()
    bnb_d = nc.dram_tensor("bnb", [BPC, HID], f32, kind="ExternalInput").ap()
    w2b_d = nc.dram_tensor("w2b", [BPC, 3, HID], f32, kind="ExternalInput").ap()
    b2b_d = nc.dram_tensor("b2b", [BPC, 3], f32, kind="ExternalInput").ap()
    out_d = nc.dram_tensor("out", [BPC, 3], f32, kind="ExternalOutput").ap()

    chbase = np.concatenate([[0], np.cumsum(nch_slots)]).astype(int)

    with tile.TileContext(nc) as tc:
        with (
            tc.tile_pool(name="singles", bufs=1) as singles,
            tc.tile_pool(name="bert_pool", bufs=3) as bert_pool,
            tc.tile_pool(name="head", bufs=1) as head,
            tc.tile_pool(name="psum_x", bufs=4, space=MemorySpace.PSUM) as psum_x_pool,
            tc.tile_pool(name="psum_p", bufs=2, space=MemorySpace.PSUM) as psum_p_pool,
            tc.tile_pool(name="psum_h", bufs=1, space=MemorySpace.PSUM) as psum_h_pool,
        ):
            # --- constant / parameter loads ---
            mask_t = singles.tile([128, ncht, nmc], bdt)
            nc.sync.dma_start(out=mask_t, in_=mask_d)
            sfac_t = singles.tile([128, BPC, nmc], f32)
            nc.sync.dma_start(out=sfac_t, in_=sfac_d)
            if PRON_FP32:
                pron_t = singles.tile([BPC, H], f32)
                nc.sync.dma_start(out=pron_t, in_=pron_d)
            bns_t = head.tile([BPC, HID], f32)
            nc.sync.dma_start(out=bns_t, in_=bns_d)
            bnb_t = head.tile([BPC, HID], f32)
            nc.sync.dma_start(out=bnb_t, in_=bnb_d)
            w2b_t = head.tile([BPC, 3, HID], f32)
            nc.sync.dma_start(out=w2b_t, in_=w2b_d)
            b2b_t = head.tile([BPC, 3], f32)
            nc.sync.dma_start(out=b2b_t, in_=b2b_d)

            # xT accumulator: [128, 3 embeddings, HC chunks * BPC batches]
            # column for contraction-chunk kc=(e*HC+hc), batch b is
            # xT[:, e, hc*BPC + b]  -> mm2 rhs slice [128, BPC] contiguous.
            xT_t = singles.tile([128, 3, HC * BPC], wdt)

            if PRON_FP32:
                # --- pron embedding: exact fp32 rows, transposed via PE ---
                from concourse.masks import make_identity
                idt = singles.tile([BPC, BPC], f32)
                make_identity(nc, idt)
                for hc in range(HC):
                    pxp = psum_p_pool.tile([128, BPC], f32, tag="pxp")
                    nc.tensor.transpose(
                        pxp, pron_t[:, hc * 128:(hc + 1) * 128], idt)
                    nc.vector.tensor_copy(
                        xT_t[:, 2, hc * BPC:(hc + 1) * BPC], pxp)

            # --- mm1: per-batch span sums (bert chunks as PE weights) ---
            # W1's DMA is interleaved into the bert stream so it lands while
            # mm1 is still consuming the tail slots.
            w1_t = singles.tile([128, KC, HID], wdt)
            for b in range(BPC):
                nch = int(nch_slots[b])
                bt = bert_pool.tile([128, nch, H], bdt, tag="bert")
                r0 = int(chbase[b]) * 128
                nc.sync.dma_start(
                    out=bt,
                    in_=bert_d[r0:r0 + nch * 128, :].rearrange(
                        "(sc p) h -> p sc h", p=128))
                if b == 3:
                    nc.sync.dma_start(
                        out=w1_t,
                        in_=w1_d.rearrange("(kc p) n -> p kc n", p=128))
                for hc in range(HC):
                    px = psum_x_pool.tile([128, nmc], f32)
                    for sc in range(nch):
                        nc.tensor.matmul(
                            px,
                            bt[:, sc, hc * 128:(hc + 1) * 128],
                            mask_t[:, int(chbase[b]) + sc, :],
                            start=(sc == 0),
                            stop=(sc == nch - 1),
                        )
                    # fp32 scale by (1/lenA, 1/lenB[, 1]) during PSUM->SBUF
                    nc.vector.tensor_mul(
                        xT_t[:, 0:nmc, hc * BPC + b], px, sfac_t[:, b, :])

            # --- mm2: h[BPC, HID] = x @ W1 (accumulate over 24 K-chunks) ---
            ph = psum_h_pool.tile([BPC, HID], f32)
            for kc in range(KC):
                e, hc = kc // HC, kc % HC
                nc.tensor.matmul(
                    ph,
                    xT_t[:, e, hc * BPC:(hc + 1) * BPC],
                    w1_t[:, kc, :],
                    start=(kc == 0),
                    stop=(kc == KC - 1),
                )

            # --- BN (eval, folded) + LeakyReLU(0.01) ---
            t1 = head.tile([BPC, HID], f32)
            nc.vector.tensor_mul(t1, ph, bns_t)
            t2 = head.tile([BPC, HID], f32)
            nc.vector.tensor_add(t2, t1, bnb_t)
            y_t = head.tile([BPC, HID], f32)
            # y = max(0.01 * t2, t2)
            nc.vector.scalar_tensor_tensor(
                y_t, t2, 0.01, t2,
                op0=mybir.AluOpType.mult, op1=mybir.AluOpType.max)

            # --- mm3: out[:, j] = b2[j] + sum(y * W2[:, j]) ---
            # (tensor_tensor_reduce crashes the exec unit on this runtime;
            # use mul + reduce_sum instead)
            osum = head.tile([BPC, 3], f32)
            scratch = head.tile([BPC, HID], f32)
            for j in range(3):
                nc.vector.tensor_mul(scratch, y_t, w2b_t[:, j, :])
                nc.vector.reduce_sum(out=osum[:, j:j + 1], in_=scratch,
                                     axis=mybir.AxisListType.X)
            o_t = head.tile([BPC, 3], f32)
            nc.vector.tensor_add(o_t, osum, b2b_t)

            nc.sync.dma_start(out=out_d, in_=o_t)

    nc.compile()
    return nc


def _prep_core_inputs(bert, bert_f32, offsets, w1, bns, bnb, w2b, b2b,
                      batch_idx, nch_slots):
    """Build the per-core input map for the given batch indices."""
    nmc = 2 if PRON_FP32 else 3
    ncht = int(sum(nch_slots))
    bertw = np.empty((ncht * 128, H), dtype=bert.dtype)
    maskp = np.zeros((128, ncht, nmc), dtype=bert.dtype)
    sfac = np.ones((BPC, nmc), dtype=np.float32)
    pron = np.empty((BPC, H), dtype=np.float32)
    row = 0
    for slot, gb in enumerate(batch_idx):
        nch = int(nch_slots[slot])
        L = nch * 128
        a0, a1, b0, b1_, p = (int(v) for v in offsets[gb])
        lo = min(a0, b0, p)
        w0 = max(0, min(lo, S - L))
        bertw[row:row + L] = bert[gb, w0:w0 + L]
        pron[slot] = bert_f32[gb, p]
        pos = w0 + np.arange(L)
        cols = [((pos >= a0) & (pos <= a1)).astype(np.float32),
                ((pos >= b0) & (pos <= b1_)).astype(np.float32)]
        if not PRON_FP32:
            cols.append((pos == p).astype(np.float32))
        sfac[slot, 0] = 1.0 / (a1 - a0 + 1)
        sfac[slot, 1] = 1.0 / (b1_ - b0 + 1)
        blk = np.stack(cols, axis=-1).reshape(nch, 128, nmc)
        maskp[:, row // 128:row // 128 + nch, :] = blk.transpose(1, 0, 2)
        row += L
    in_map = {
        "bertw": bertw,
        "maskp": maskp,
        "sfac": np.broadcast_to(sfac, (128, BPC, nmc)).copy(),
        "w1": w1,
        "bns": bns,
        "bnb": bnb,
        "w2b": w2b,
        "b2b": b2b,
    }
    if PRON_FP32:
        in_map["pron"] = pron
    return in_map


def kernel(bert_outputs, offsets, W1, b1, gamma, beta, running_mean,
           running_var, W2, b2):
    import ml_dtypes

    bert_f32 = np.ascontiguousarray(np.asarray(bert_outputs, dtype=np.float32))
    bert = bert_f32.astype(ml_dtypes.bfloat16) if BERT_BF16 else bert_f32
    offs = np.asarray(offsets).astype(np.int64)
    W1 = np.ascontiguousarray(np.asarray(W1, dtype=np.float32))
    if W1_BF16:
        W1 = W1.astype(ml_dtypes.bfloat16)
    b1 = np.asarray(b1, dtype=np.float32)
    gamma = np.asarray(gamma, dtype=np.float32)
    beta = np.asarray(beta, dtype=np.float32)
    rm = np.asarray(running_mean, dtype=np.float32)
    rv = np.asarray(running_var, dtype=np.float32)
    W2 = np.asarray(W2, dtype=np.float32)
    b2 = np.asarray(b2, dtype=np.float32)

    # Fold BN eval-mode stats: bn(xW1 + b1) = (xW1)*s + ((b1 - mean)*s + beta)
    s = gamma / np.sqrt(rv + EPS)
    bias = (b1 - rm) * s + beta
    bns = np.broadcast_to(s, (BPC, HID)).copy()
    bnb = np.broadcast_to(bias, (BPC, HID)).copy()
    w2b = np.broadcast_to(W2.T[None, :, :], (BPC, 3, HID)).copy()
    b2b = np.broadcast_to(b2, (BPC, 3)).copy()

    # Row windows: union of span/pron rows per batch, padded to 128-row
    # chunks. Sort batches by window size so same-slot batches across cores
    # share one (max) chunk count; undone at gather time.
    if WINDOW:
        lo = offs[:, [0, 2, 4]].min(axis=1)
        hi = offs[:, [1, 3, 4]].max(axis=1)
        lens = np.minimum((hi - lo + 128) // 128 * 128, S)
        order = np.argsort(-lens, kind="stable")
    else:
        lens = np.full(B, S, dtype=np.int64)
        order = np.arange(B)

    # slot i of every core holds batches ranked [i*NCORES, (i+1)*NCORES)
    perm = order.reshape(BPC, NCORES)  # [slot, core] -> global batch
    nch_slots = tuple(int(lens[perm[i]].max()) // 128 for i in range(BPC))

    key = nch_slots
    if key not in _PROGRAM_CACHE:
        _PROGRAM_CACHE[key] = _build_program(key)
    nc = _PROGRAM_CACHE[key]

    in_maps = [
        _prep_core_inputs(bert, bert_f32, offs, W1, bns, bnb, w2b, b2b,
                          perm[:, c], nch_slots)
        for c in range(NCORES)
    ]

    from concourse import bass_utils
    kwargs = {}
    if TRACE:
        kwargs = {"trace": True, "trace_cores": list(range(NCORES))}
    res = bass_utils.run_bass_kernel_spmd(nc, in_maps,
                                          core_ids=list(range(NCORES)),
                                          **kwargs)
    global LAST_RESULT
    LAST_RESULT = res

    out = np.empty((B, 3), dtype=np.float32)
    for c in range(NCORES):
        out[perm[:, c]] = res.results[c]["out"]
    return out


# revision 33
# speedup vs baseline: 2.4870x; 1.0163x over previous
"""CorefHead Trainium2 kernel.

Reference computation (B=64, S=512, H=1024, HID=512):
  emb_a = span_mean(bert, offsets[:,0:2])   # [B,H]
  emb_b = span_mean(bert, offsets[:,2:4])   # [B,H]
  emb_p = bert[b, offsets[:,4]]             # [B,H]
  x = concat([emb_a, emb_b, emb_p], -1)     # [B,3H]
  h = leaky_relu(batchnorm_eval(x @ W1 + b1), 0.01)
  out = h @ W2 + b2                         # [B,3]

Strategy: pure data parallel, batch sharded 8 ways (8 batches/core).
Per core:
  - Host precomputes prescaled span masks (1/len) + pron one-hot, packed
    [128, nch, 3]; host also packs only the needed row-window of bert per
    batch (union of span/pron rows, padded to 128-row chunks).
  - mm1 (PE): xT[h_chunk] += bert_chunk.T @ mask_chunk -> x transposed
    [3072, 8] directly (no on-device transpose needed).
  - mm2 (PE): h[8, 512] += xT_chunk.T @ W1_chunk over 24 K-chunks.
  - BN+LeakyReLU (DVE): y = max(t, 0.01*t), t = h*scale + bias with
    scale/bias folded from (b1, gamma, beta, running stats) on host.
  - mm3 (DVE): out[:, j] = b2[j] + sum(y * W2[:, j]) via tensor_tensor_reduce.
Host gathers per-core [8, 3] outputs and undoes the batch permutation.
"""

import numpy as np

B, S, H = 64, 512, 1024
HID = 512
EPS = 1e-5
NCORES = 8
BPC = B // NCORES  # batches per core
KC = 3 * H // 128  # 24 contraction chunks for mm2
HC = H // 128      # 8 h-chunks per embedding

# Set to True to ship only the needed row-window of bert per batch.
WINDOW = True
# bert/masks in bf16: halves the dominant DMA traffic and runs mm1 single
# pass with fast weight load on the PE. Masks hold exact 0/1 (bf16-exact);
# the 1/span_len scaling happens in fp32 on the PSUM->SBUF copy.
BERT_BF16 = True
# W1 (and the mm2 x operand) in bf16.
W1_BF16 = True
# Ship pron rows fp32 + transpose on device. Only buys precision when the
# mm2 operands stay fp32 (otherwise x is rounded to bf16 anyway).
PRON_FP32 = not W1_BF16

# Test-harness hooks (harness calls kernel() with TRACE=False default).
TRACE = False
LAST_RESULT = None

_PROGRAM_CACHE: dict = {}


def _build_program(nch_slots: tuple):
    """Build + compile the SPMD Bass program for the given per-slot chunk
    counts (number of 128-row S-chunks shipped per batch slot)."""
    import concourse.bacc as bacc
    import concourse.tile as tile
    import concourse.mybir as mybir
    from concourse.bass import MemorySpace

    f32 = mybir.dt.float32
    bdt = mybir.dt.bfloat16 if BERT_BF16 else f32
    wdt = mybir.dt.bfloat16 if W1_BF16 else f32
    ncht = int(sum(nch_slots))
    totrows = ncht * 128

    nc = bacc.Bacc("TRN2", target_bir_lowering=False, debug=False,
                   num_devices=NCORES)

    nmc = 2 if PRON_FP32 else 3  # mask columns (spans only, or spans+pron)

    bert_d = nc.dram_tensor("bertw", [totrows, H], bdt, kind="ExternalInput").ap()
    mask_d = nc.dram_tensor("maskp", [128, ncht, nmc], bdt, kind="ExternalInput").ap()
    sfac_d = nc.dram_tensor("sfac", [128, BPC, nmc], f32, kind="ExternalInput").ap()
    if PRON_FP32:
        pron_d = nc.dram_tensor("pron", [BPC, H], f32, kind="ExternalInput").ap()
    w1_d = nc.dram_tensor("w1", [3 * H, HID], wdt, kind="ExternalInput").ap()
    # bnbP[p, mc] = folded BN bias for hid index mc*128+p; w2P[p, mc, j] =
    # W2[mc*128+p, j]; b2c = b2[:, None]
    bnb_d = nc.dram_tensor("bnbP", [128, HID // 128], f32,
                           kind="ExternalInput").ap()
    w2_d = nc.dram_tensor("w2P", [128, HID // 128, 3], f32,
                          kind="ExternalInput").ap()
    b2_d = nc.dram_tensor("b2c", [3, 1], f32, kind="ExternalInput").ap()
    out_d = nc.dram_tensor("out", [3, BPC], f32, kind="ExternalOutput").ap()

    chbase = np.concatenate([[0], np.cumsum(nch_slots)]).astype(int)

    with tile.TileContext(nc) as tc:
        with (
            tc.tile_pool(name="singles", bufs=1) as singles,
            tc.tile_pool(name="bert_pool", bufs=3) as bert_pool,
            tc.tile_pool(name="head", bufs=1) as head,
            tc.tile_pool(name="psum_x", bufs=4, space=MemorySpace.PSUM) as psum_x_pool,
            tc.tile_pool(name="psum_p", bufs=2, space=MemorySpace.PSUM) as psum_p_pool,
            tc.tile_pool(name="psum_h", bufs=1, space=MemorySpace.PSUM) as psum_h_pool,
        ):
            # --- constant / parameter loads ---
            mask_t = singles.tile([128, ncht, nmc], bdt)
            nc.sync.dma_start(out=mask_t, in_=mask_d)
            sfac_t = singles.tile([128, BPC, nmc], f32)
            nc.sync.dma_start(out=sfac_t, in_=sfac_d)
            if PRON_FP32:
                pron_t = singles.tile([BPC, H], f32)
                nc.sync.dma_start(out=pron_t, in_=pron_d)
            bnb_t = head.tile([128, HID // 128], f32)
            nc.sync.dma_start(out=bnb_t, in_=bnb_d)
            w2_t = head.tile([128, HID // 128, 3], f32)
            nc.sync.dma_start(out=w2_t, in_=w2_d)
            b2_t = head.tile([3, 1], f32)
            nc.sync.dma_start(out=b2_t, in_=b2_d)
            from concourse.masks import make_identity
            idt = singles.tile([BPC, BPC], f32)
            make_identity(nc, idt)

            # xT accumulator: [128, 3 embeddings, HC chunks * BPC batches]
            # column for contraction-chunk kc=(e*HC+hc), batch b is
            # xT[:, e, hc*BPC + b]  -> mm2 rhs slice [128, BPC] contiguous.
            xT_t = singles.tile([128, 3, HC * BPC], wdt)

            if PRON_FP32:
                # --- pron embedding: exact fp32 rows, transposed via PE ---
                for hc in range(HC):
                    pxp = psum_p_pool.tile([128, BPC], f32, tag="pxp")
                    nc.tensor.transpose(
                        pxp, pron_t[:, hc * 128:(hc + 1) * 128], idt)
                    nc.vector.tensor_copy(
                        xT_t[:, 2, hc * BPC:(hc + 1) * BPC], pxp)

            # --- mm1: per-batch span sums (bert chunks as PE weights) ---
            # W1's DMA is interleaved into the bert stream so it lands while
            # mm1 is still consuming the tail slots.
            w1_t = singles.tile([128, KC, HID], wdt)
            for b in range(BPC):
                nch = int(nch_slots[b])
                bt = bert_pool.tile([128, nch, H], bdt, tag="bert")
                r0 = int(chbase[b]) * 128
                nc.sync.dma_start(
                    out=bt,
                    in_=bert_d[r0:r0 + nch * 128, :].rearrange(
                        "(sc p) h -> p sc h", p=128))
                if b == 3:
                    nc.sync.dma_start(
                        out=w1_t,
                        in_=w1_d.rearrange("(kc p) n -> p kc n", p=128))
                for hc in range(HC):
                    px = psum_x_pool.tile([128, nmc], f32)
                    for sc in range(nch):
                        nc.tensor.matmul(
                            px,
                            bt[:, sc, hc * 128:(hc + 1) * 128],
                            mask_t[:, int(chbase[b]) + sc, :],
                            start=(sc == 0),
                            stop=(sc == nch - 1),
                        )
                    # fp32 scale by (1/lenA, 1/lenB[, 1]) during PSUM->SBUF
                    nc.vector.tensor_mul(
                        xT_t[:, 0:nmc, hc * BPC + b], px, sfac_t[:, b, :])

            # --- mm2: h[BPC, HID] = x @ (W1 * bn_scale) over 24 K-chunks ---
            # (the BN eval-mode scale is folded into W1 on the host)
            ph = psum_h_pool.tile([BPC, HID], f32)
            for kc in range(KC):
                e, hc = kc // HC, kc % HC
                nc.tensor.matmul(
                    ph,
                    xT_t[:, e, hc * BPC:(hc + 1) * BPC],
                    w1_t[:, kc, :],
                    start=(kc == 0),
                    stop=(kc == KC - 1),
                )
            hs_t = head.tile([BPC, HID], f32)
            nc.vector.tensor_copy(hs_t, ph)

            # --- per hid-chunk: transpose h, + BN bias, LeakyReLU, mm3 ---
            ot_ps = psum_h_pool.tile([3, BPC], f32, tag="oT")
            for mc in range(HID // 128):
                pht = psum_p_pool.tile([128, BPC], f32, tag="pht")
                nc.tensor.transpose(
                    pht, hs_t[:, mc * 128:(mc + 1) * 128], idt)
                t_t = head.tile([128, BPC], f32, tag="t_t")
                nc.vector.tensor_scalar_add(t_t, pht, bnb_t[:, mc:mc + 1])
                y_t = head.tile([128, BPC], f32, tag="y_t")
                # y = max(0.01 * t, t)
                nc.vector.scalar_tensor_tensor(
                    y_t, t_t, 0.01, t_t,
                    op0=mybir.AluOpType.mult, op1=mybir.AluOpType.max)
                nc.tensor.matmul(
                    ot_ps, w2_t[:, mc, :], y_t,
                    start=(mc == 0), stop=(mc == HID // 128 - 1))

            o_t = head.tile([3, BPC], f32)
            nc.vector.tensor_scalar_add(o_t, ot_ps, b2_t)
            nc.sync.dma_start(out=out_d, in_=o_t)

    nc.compile()
    return nc


def _prep_core_inputs(bert, bert_f32, offsets, w1, bnbP, w2P, b2c,
                      batch_idx, nch_slots):
    """Build the per-core input map for the given batch indices."""
    nmc = 2 if PRON_FP32 else 3
    ncht = int(sum(nch_slots))
    bertw = np.empty((ncht * 128, H), dtype=bert.dtype)
    maskp = np.zeros((128, ncht, nmc), dtype=bert.dtype)
    sfac = np.ones((BPC, nmc), dtype=np.float32)
    pron = np.empty((BPC, H), dtype=np.float32)
    row = 0
    for slot, gb in enumerate(batch_idx):
        nch = int(nch_slots[slot])
        L = nch * 128
        a0, a1, b0, b1_, p = (int(v) for v in offsets[gb])
        lo = min(a0, b0, p)
        w0 = max(0, min(lo, S - L))
        bertw[row:row + L] = bert[gb, w0:w0 + L]
        pron[slot] = bert_f32[gb, p]
        pos = w0 + np.arange(L)
        cols = [((pos >= a0) & (pos <= a1)).astype(np.float32),
                ((pos >= b0) & (pos <= b1_)).astype(np.float32)]
        if not PRON_FP32:
            cols.append((pos == p).astype(np.float32))
        sfac[slot, 0] = 1.0 / (a1 - a0 + 1)
        sfac[slot, 1] = 1.0 / (b1_ - b0 + 1)
        blk = np.stack(cols, axis=-1).reshape(nch, 128, nmc)
        maskp[:, row // 128:row // 128 + nch, :] = blk.transpose(1, 0, 2)
        row += L
    in_map = {
        "bertw": bertw,
        "maskp": maskp,
        "sfac": np.broadcast_to(sfac, (128, BPC, nmc)).copy(),
        "w1": w1,
        "bnbP": bnbP,
        "w2P": w2P,
        "b2c": b2c,
    }
    if PRON_FP32:
        in_map["pron"] = pron
    return in_map


def kernel(bert_outputs, offsets, W1, b1, gamma, beta, running_mean,
           running_var, W2, b2):
    import ml_dtypes

    bert_f32 = np.ascontiguousarray(np.asarray(bert_outputs, dtype=np.float32))
    bert = bert_f32.astype(ml_dtypes.bfloat16) if BERT_BF16 else bert_f32
    offs = np.asarray(offsets).astype(np.int64)
    W1 = np.asarray(W1, dtype=np.float32)
    b1 = np.asarray(b1, dtype=np.float32)
    gamma = np.asarray(gamma, dtype=np.float32)
    beta = np.asarray(beta, dtype=np.float32)
    rm = np.asarray(running_mean, dtype=np.float32)
    rv = np.asarray(running_var, dtype=np.float32)
    W2 = np.asarray(W2, dtype=np.float32)
    b2 = np.asarray(b2, dtype=np.float32)

    # Fold BN eval-mode stats: bn(xW1 + b1) = x(W1*s) + ((b1 - mean)*s + beta)
    s = gamma / np.sqrt(rv + EPS)
    bias = (b1 - rm) * s + beta
    W1 = np.ascontiguousarray(W1 * s[None, :])
    if W1_BF16:
        W1 = W1.astype(ml_dtypes.bfloat16)
    bnbP = np.ascontiguousarray(bias.reshape(HID // 128, 128).T)
    w2P = np.ascontiguousarray(
        W2.reshape(HID // 128, 128, 3).transpose(1, 0, 2))
    b2c = np.ascontiguousarray(b2.reshape(3, 1))

    # Row windows: union of span/pron rows per batch, padded to 128-row
    # chunks. Sort batches by window size so same-slot batches across cores
    # share one (max) chunk count; undone at gather time.
    if WINDOW:
        lo = offs[:, [0, 2, 4]].min(axis=1)
        hi = offs[:, [1, 3, 4]].max(axis=1)
        lens = np.minimum((hi - lo + 128) // 128 * 128, S)
        order = np.argsort(-lens, kind="stable")
    else:
        lens = np.full(B, S, dtype=np.int64)
        order = np.arange(B)

    # slot i of every core holds batches ranked [i*NCORES, (i+1)*NCORES)
    perm = order.reshape(BPC, NCORES)  # [slot, core] -> global batch
    nch_slots = tuple(int(lens[perm[i]].max()) // 128 for i in range(BPC))

    key = nch_slots
    if key not in _PROGRAM_CACHE:
        _PROGRAM_CACHE[key] = _build_program(key)
    nc = _PROGRAM_CACHE[key]

    in_maps = [
        _prep_core_inputs(bert, bert_f32, offs, W1, bnbP, w2P, b2c,
                          perm[:, c], nch_slots)
        for c in range(NCORES)
    ]

    from concourse import bass_utils
    kwargs = {}
    if TRACE:
        kwargs = {"trace": True, "trace_cores": list(range(NCORES))}
    res = bass_utils.run_bass_kernel_spmd(nc, in_maps,
                                          core_ids=list(range(NCORES)),
                                          **kwargs)
    global LAST_RESULT
    LAST_RESULT = res

    out = np.empty((B, 3), dtype=np.float32)
    for c in range(NCORES):
        out[perm[:, c]] = res.results[c]["out"].T
    return out


# revision 34
# speedup vs baseline: 2.5872x; 1.0403x over previous
"""CorefHead Trainium2 kernel.

Reference computation (B=64, S=512, H=1024, HID=512):
  emb_a = span_mean(bert, offsets[:,0:2])   # [B,H]
  emb_b = span_mean(bert, offsets[:,2:4])   # [B,H]
  emb_p = bert[b, offsets[:,4]]             # [B,H]
  x = concat([emb_a, emb_b, emb_p], -1)     # [B,3H]
  h = leaky_relu(batchnorm_eval(x @ W1 + b1), 0.01)
  out = h @ W2 + b2                         # [B,3]

Strategy: pure data parallel, batch sharded 8 ways (8 batches/core).
Per core:
  - Host precomputes prescaled span masks (1/len) + pron one-hot, packed
    [128, nch, 3]; host also packs only the needed row-window of bert per
    batch (union of span/pron rows, padded to 128-row chunks).
  - mm1 (PE): xT[h_chunk] += bert_chunk.T @ mask_chunk -> x transposed
    [3072, 8] directly (no on-device transpose needed).
  - mm2 (PE): h[8, 512] += xT_chunk.T @ W1_chunk over 24 K-chunks.
  - BN+LeakyReLU (DVE): y = max(t, 0.01*t), t = h*scale + bias with
    scale/bias folded from (b1, gamma, beta, running stats) on host.
  - mm3 (DVE): out[:, j] = b2[j] + sum(y * W2[:, j]) via tensor_tensor_reduce.
Host gathers per-core [8, 3] outputs and undoes the batch permutation.
"""

import numpy as np

B, S, H = 64, 512, 1024
HID = 512
EPS = 1e-5
NCORES = 8
BPC = B // NCORES  # batches per core
KC = 3 * H // 128  # 24 contraction chunks for mm2
HC = H // 128      # 8 h-chunks per embedding

# Set to True to ship only the needed row-window of bert per batch.
WINDOW = True
# bert/masks in bf16: halves the dominant DMA traffic and runs mm1 single
# pass with fast weight load on the PE. Masks hold exact 0/1 (bf16-exact);
# the 1/span_len scaling happens in fp32 on the PSUM->SBUF copy.
BERT_BF16 = True
# W1 (and the mm2 x operand) in bf16.
W1_BF16 = True
# Ship pron rows fp32 + transpose on device. Only buys precision when the
# mm2 operands stay fp32 (otherwise x is rounded to bf16 anyway).
PRON_FP32 = not W1_BF16

# Test-harness hooks (harness calls kernel() with TRACE=False default).
TRACE = False
LAST_RESULT = None

_PROGRAM_CACHE: dict = {}


def _build_program(nch_slots: tuple):
    """Build + compile the SPMD Bass program for the given per-slot chunk
    counts (number of 128-row S-chunks shipped per batch slot)."""
    import concourse.bacc as bacc
    import concourse.tile as tile
    import concourse.mybir as mybir
    from concourse.bass import MemorySpace

    f32 = mybir.dt.float32
    bdt = mybir.dt.bfloat16 if BERT_BF16 else f32
    wdt = mybir.dt.bfloat16 if W1_BF16 else f32
    ncht = int(sum(nch_slots))
    totrows = ncht * 128

    nc = bacc.Bacc("TRN2", target_bir_lowering=False, debug=False,
                   num_devices=NCORES)

    nmc = 2 if PRON_FP32 else 3  # mask columns (spans only, or spans+pron)

    bert_d = nc.dram_tensor("bertw", [totrows, H], bdt, kind="ExternalInput").ap()
    mask_d = nc.dram_tensor("maskp", [128, ncht, nmc], bdt, kind="ExternalInput").ap()
    sfac_d = nc.dram_tensor("sfac", [128, BPC, nmc], f32, kind="ExternalInput").ap()
    if PRON_FP32:
        pron_d = nc.dram_tensor("pron", [BPC, H], f32, kind="ExternalInput").ap()
    w1_d = nc.dram_tensor("w1", [3 * H, HID], wdt, kind="ExternalInput").ap()
    # bnbP[p, mc] = folded BN bias for hid index mc*128+p; w2P[p, mc, j] =
    # W2[mc*128+p, j]; b2c = b2[:, None]
    bnb_d = nc.dram_tensor("bnbP", [128, HID // 128], f32,
                           kind="ExternalInput").ap()
    w2_d = nc.dram_tensor("w2P", [128, HID // 128, 3], f32,
                          kind="ExternalInput").ap()
    b2_d = nc.dram_tensor("b2c", [3, 1], f32, kind="ExternalInput").ap()
    out_d = nc.dram_tensor("out", [3, BPC], f32, kind="ExternalOutput").ap()

    chbase = np.concatenate([[0], np.cumsum(nch_slots)]).astype(int)

    with tile.TileContext(nc) as tc:
        with (
            tc.tile_pool(name="singles", bufs=1) as singles,
            tc.tile_pool(name="bert_pool", bufs=3) as bert_pool,
            tc.tile_pool(name="head", bufs=1) as head,
            tc.tile_pool(name="psum_x", bufs=4, space=MemorySpace.PSUM) as psum_x_pool,
            tc.tile_pool(name="psum_p", bufs=2, space=MemorySpace.PSUM) as psum_p_pool,
            tc.tile_pool(name="psum_h", bufs=1, space=MemorySpace.PSUM) as psum_h_pool,
        ):
            # --- constant / parameter loads ---
            mask_t = singles.tile([128, ncht, nmc], bdt)
            nc.scalar.dma_start(out=mask_t, in_=mask_d)
            sfac_t = singles.tile([128, BPC, nmc], f32)
            nc.scalar.dma_start(out=sfac_t, in_=sfac_d)
            if PRON_FP32:
                pron_t = singles.tile([BPC, H], f32)
                nc.scalar.dma_start(out=pron_t, in_=pron_d)
            bnb_t = head.tile([128, HID // 128], f32)
            nc.scalar.dma_start(out=bnb_t, in_=bnb_d)
            w2_t = head.tile([128, HID // 128, 3], f32)
            nc.scalar.dma_start(out=w2_t, in_=w2_d)
            b2_t = head.tile([3, 1], f32)
            nc.scalar.dma_start(out=b2_t, in_=b2_d)
            from concourse.masks import make_identity
            idt = singles.tile([BPC, BPC], f32)
            make_identity(nc, idt)

            # xT accumulator: [128, 3 embeddings, HC chunks * BPC batches]
            # column for contraction-chunk kc=(e*HC+hc), batch b is
            # xT[:, e, hc*BPC + b]  -> mm2 rhs slice [128, BPC] contiguous.
            xT_t = singles.tile([128, 3, HC * BPC], wdt)

            if PRON_FP32:
                # --- pron embedding: exact fp32 rows, transposed via PE ---
                for hc in range(HC):
                    pxp = psum_p_pool.tile([128, BPC], f32, tag="pxp")
                    nc.tensor.transpose(
                        pxp, pron_t[:, hc * 128:(hc + 1) * 128], idt)
                    nc.vector.tensor_copy(
                        xT_t[:, 2, hc * BPC:(hc + 1) * BPC], pxp)

            # --- mm1: per-batch span sums (bert chunks as PE weights) ---
            # W1 + consts ride the ACT HWDGE ring; berts ride the SP ring.
            # The SDMA engines round-robin between the rings, so the bert
            # stream is never stuck behind the big W1 transfer.
            w1_t = singles.tile([128, KC, HID], wdt)
            w1_src = w1_d.rearrange("(kc p) n -> p kc n", p=128)
            for i in range(4):
                nc.scalar.dma_start(
                    out=w1_t[:, 6 * i:6 * (i + 1), :],
                    in_=w1_src[:, 6 * i:6 * (i + 1), :])
            for b in range(BPC):
                nch = int(nch_slots[b])
                bt = bert_pool.tile([128, nch, H], bdt, tag="bert")
                r0 = int(chbase[b]) * 128
                nc.sync.dma_start(
                    out=bt,
                    in_=bert_d[r0:r0 + nch * 128, :].rearrange(
                        "(sc p) h -> p sc h", p=128))
                for hc in range(HC):
                    px = psum_x_pool.tile([128, nmc], f32)
                    for sc in range(nch):
                        nc.tensor.matmul(
                            px,
                            bt[:, sc, hc * 128:(hc + 1) * 128],
                            mask_t[:, int(chbase[b]) + sc, :],
                            start=(sc == 0),
                            stop=(sc == nch - 1),
                        )
                    # fp32 scale by (1/lenA, 1/lenB[, 1]) during PSUM->SBUF
                    nc.vector.tensor_mul(
                        xT_t[:, 0:nmc, hc * BPC + b], px, sfac_t[:, b, :])

            # --- mm2: h[BPC, HID] = x @ (W1 * bn_scale) over 24 K-chunks ---
            # (the BN eval-mode scale is folded into W1 on the host)
            ph = psum_h_pool.tile([BPC, HID], f32)
            for kc in range(KC):
                e, hc = kc // HC, kc % HC
                nc.tensor.matmul(
                    ph,
                    xT_t[:, e, hc * BPC:(hc + 1) * BPC],
                    w1_t[:, kc, :],
                    start=(kc == 0),
                    stop=(kc == KC - 1),
                )
            hs_t = head.tile([BPC, HID], f32)
            nc.vector.tensor_copy(hs_t, ph)

            # --- per hid-chunk: transpose h, + BN bias, LeakyReLU, mm3 ---
            ot_ps = psum_h_pool.tile([3, BPC], f32, tag="oT")
            for mc in range(HID // 128):
                pht = psum_p_pool.tile([128, BPC], f32, tag="pht")
                nc.tensor.transpose(
                    pht, hs_t[:, mc * 128:(mc + 1) * 128], idt)
                t_t = head.tile([128, BPC], f32, tag="t_t")
                nc.vector.tensor_scalar_add(t_t, pht, bnb_t[:, mc:mc + 1])
                y_t = head.tile([128, BPC], f32, tag="y_t")
                # y = max(0.01 * t, t)
                nc.vector.scalar_tensor_tensor(
                    y_t, t_t, 0.01, t_t,
                    op0=mybir.AluOpType.mult, op1=mybir.AluOpType.max)
                nc.tensor.matmul(
                    ot_ps, w2_t[:, mc, :], y_t,
                    start=(mc == 0), stop=(mc == HID // 128 - 1))

            o_t = head.tile([3, BPC], f32)
            nc.vector.tensor_scalar_add(o_t, ot_ps, b2_t)
            nc.sync.dma_start(out=out_d, in_=o_t)

    nc.compile()
    return nc


def _prep_core_inputs(bert, bert_f32, offsets, w1, bnbP, w2P, b2c,
                      batch_idx, nch_slots):
    """Build the per-core input map for the given batch indices."""
    nmc = 2 if PRON_FP32 else 3
    ncht = int(sum(nch_slots))
    bertw = np.empty((ncht * 128, H), dtype=bert.dtype)
    maskp = np.zeros((128, ncht, nmc), dtype=bert.dtype)
    sfac = np.ones((BPC, nmc), dtype=np.float32)
    pron = np.empty((BPC, H), dtype=np.float32)
    row = 0
    for slot, gb in enumerate(batch_idx):
        nch = int(nch_slots[slot])
        L = nch * 128
        a0, a1, b0, b1_, p = (int(v) for v in offsets[gb])
        lo = min(a0, b0, p)
        w0 = max(0, min(lo, S - L))
        bertw[row:row + L] = bert[gb, w0:w0 + L]
        pron[slot] = bert_f32[gb, p]
        pos = w0 + np.arange(L)
        cols = [((pos >= a0) & (pos <= a1)).astype(np.float32),
                ((pos >= b0) & (pos <= b1_)).astype(np.float32)]
        if not PRON_FP32:
            cols.append((pos == p).astype(np.float32))
        sfac[slot, 0] = 1.0 / (a1 - a0 + 1)
        sfac[slot, 1] = 1.0 / (b1_ - b0 + 1)
        blk = np.stack(cols, axis=-1).reshape(nch, 128, nmc)
        maskp[:, row // 128:row // 128 + nch, :] = blk.transpose(1, 0, 2)
        row += L
    in_map = {
        "bertw": bertw,
        "maskp": maskp,
        "sfac": np.broadcast_to(sfac, (128, BPC, nmc)).copy(),
        "w1": w1,
        "bnbP": bnbP,
        "w2P": w2P,
        "b2c": b2c,
    }
    if PRON_FP32:
        in_map["pron"] = pron
    return in_map


def kernel(bert_outputs, offsets, W1, b1, gamma, beta, running_mean,
           running_var, W2, b2):
    import ml_dtypes

    bert_f32 = np.ascontiguousarray(np.asarray(bert_outputs, dtype=np.float32))
    bert = bert_f32.astype(ml_dtypes.bfloat16) if BERT_BF16 else bert_f32
    offs = np.asarray(offsets).astype(np.int64)
    W1 = np.asarray(W1, dtype=np.float32)
    b1 = np.asarray(b1, dtype=np.float32)
    gamma = np.asarray(gamma, dtype=np.float32)
    beta = np.asarray(beta, dtype=np.float32)
    rm = np.asarray(running_mean, dtype=np.float32)
    rv = np.asarray(running_var, dtype=np.float32)
    W2 = np.asarray(W2, dtype=np.float32)
    b2 = np.asarray(b2, dtype=np.float32)

    # Fold BN eval-mode stats: bn(xW1 + b1) = x(W1*s) + ((b1 - mean)*s + beta)
    s = gamma / np.sqrt(rv + EPS)
    bias = (b1 - rm) * s + beta
    W1 = np.ascontiguousarray(W1 * s[None, :])
    if W1_BF16:
        W1 = W1.astype(ml_dtypes.bfloat16)
    bnbP = np.ascontiguousarray(bias.reshape(HID // 128, 128).T)
    w2P = np.ascontiguousarray(
        W2.reshape(HID // 128, 128, 3).transpose(1, 0, 2))
    b2c = np.ascontiguousarray(b2.reshape(3, 1))

    # Row windows: union of span/pron rows per batch, padded to 128-row
    # chunks. Sort batches by window size so same-slot batches across cores
    # share one (max) chunk count; undone at gather time.
    if WINDOW:
        lo = offs[:, [0, 2, 4]].min(axis=1)
        hi = offs[:, [1, 3, 4]].max(axis=1)
        lens = np.minimum((hi - lo + 128) // 128 * 128, S)
        order = np.argsort(-lens, kind="stable")
    else:
        lens = np.full(B, S, dtype=np.int64)
        order = np.arange(B)

    # slot i of every core holds batches ranked [i*NCORES, (i+1)*NCORES)
    perm = order.reshape(BPC, NCORES)  # [slot, core] -> global batch
    nch_slots = tuple(int(lens[perm[i]].max()) // 128 for i in range(BPC))

    key = nch_slots
    if key not in _PROGRAM_CACHE:
        _PROGRAM_CACHE[key] = _build_program(key)
    nc = _PROGRAM_CACHE[key]

    in_maps = [
        _prep_core_inputs(bert, bert_f32, offs, W1, bnbP, w2P, b2c,
                          perm[:, c], nch_slots)
        for c in range(NCORES)
    ]

    from concourse import bass_utils
    kwargs = {}
    if TRACE:
        kwargs = {"trace": True, "trace_cores": list(range(NCORES))}
    res = bass_utils.run_bass_kernel_spmd(nc, in_maps,
                                          core_ids=list(range(NCORES)),
                                          **kwargs)
    global LAST_RESULT
    LAST_RESULT = res

    out = np.empty((B, 3), dtype=np.float32)
    for c in range(NCORES):
        out[perm[:, c]] = res.results[c]["out"].T
    return out
